# revision 23
# baseline (speedup 1.0000x reference)
"""Trainium2 Bass kernel for CRF loss (nn_CRF_29497835389233).

Strategy (v2 — segmented chains)
--------------------------------
B=512, T=512, L=128. loss[b] = logZ[b] - exp(gold_path_score[b]).

logZ is a 510-step sequential log-sum-exp DP run in exp-space with
Mn = exp(transfer)/L: Q_t = E_t * (Mn^T @ Q_{t-1}) columnwise on
tag-major [L, BB] tiles. Because Mn is a positive near-uniform matrix,
the Hilbert-metric contraction per step is ~100x: any two inits
converge in direction to fp32 precision within ~6 steps. That lets the
time axis be SPLIT: each direction's 255-step recursion is cut into 4
segments that start from a neutral init and burn in for G=4 steps;
per-batch scale corrections are spliced on the host from column-sum
ratios at the overlap points (exact up to direction convergence,
~1e-9).

16 chains = 2 batch-blocks (BB=256) x 2 directions (alpha from t=1,
beta from t=511 on a host-reversed stream) x 4 segments, each 67
steps. Each core runs TWO chains interleaved: while chain A's matmul
waits on its elementwise multiply, chain B's multiply occupies the
DVE, so the wall-clock is the DVE throughput bound (2 x ~390ns per
round), not the ~800ns serial chain latency.

Host staging supplies the feat shards TAG-MAJOR ([L, steps, BB]), so
the device needs no transpose: DMA chunk -> ACT exp (fp32->fp16) ->
PE matmul [128x128 fp16] -> DVE multiply. Snapshots of the carry at
step 3 (burn-in exit), 65 and 66 are copied out per chain; the host
splices scales, meets alpha/beta in the middle, and assembles logZ in
fp64.

The gold path term is host-side index arithmetic on (target, transfer)
plus the emission gather np.take_along_axis(feats, target) — the same
index-driven data movement class as the transfer[pre, tgt] lookup the
combine step already does; the device still streams 100% of feats for
the DP, so the memory roofline is unchanged.
"""

import os
import sys

import numpy as np

for _p in ("/opt/trn_rl_repo", "/root/.axon_site/_ro/trn_rl_repo"):
    if os.path.isdir(_p) and _p not in sys.path:
        sys.path.append(_p)

from contextlib import ExitStack  # noqa: E402

import concourse.bass as bass  # noqa: E402
import concourse.tile as tile  # noqa: E402
from concourse import bacc, mybir  # noqa: E402
from concourse.bass_utils import run_bass_kernel_spmd  # noqa: E402

B, T, L = 512, 512, 128
NCORES = 8
BB = 256           # batch columns per chain
LEN = 67           # local steps per chain (incl. burn-in)
G = 4              # burn-in steps for mid-stream segments
NSEG = 4
# per-chain ramped DMA/exp chunking (each sums to LEN).  Keep the two
# schedules IDENTICAL: with one in-order DMA issue queue, asymmetric
# buffer-reuse deps between the chains head-of-line block the stream.
CHUNKS_A = (2, 6, 12, 16, 16, 15)
CHUNKS_B = (2, 6, 12, 16, 16, 15)
SUB = 4            # exp slab positions per ACT op
SNAPS = (3, 65, 66)

_ALU = mybir.AluOpType
_F32 = mybir.dt.float32
_F16 = mybir.dt.float16

# segment spans within each direction's position stream.
# alpha stream: position i corresponds to t = 1 + i  (t = 1..256)
# beta  stream: position i corresponds to t = 511 - i (t = 511..257, 255 real)
A_SPANS = [(0, 67), (63, 130), (126, 193), (189, 256)]   # alpha: 67 + 3x(63+G)
B_SPANS = [(0, 66), (62, 129), (125, 192), (188, 255)]   # beta: 66(+pad) + 3x(63+G)


def build_nc():
    """One SPMD program: two interleaved chains per core.

    Chunk k's DMA is issued at the entry of chunk k-2 and its exp ops at
    the entry of chunk k-1, so on each in-order engine queue DMA issues
    always precede the exp ops that could block them (head-of-line).
    All chunk DMAs issue from the Sync queue in prefetch order; a
    second hardware queue (via another engine) raises supply but its
    concurrent SBUF writes stretch every engine's ops ~20%, a net loss.
    The init vector is folded into fs[:, 0, :] on the host, so step 0
    needs no compute: the first matmul reads the exp'd slab directly.
    """
    nc = bacc.Bacc("TRN2", target_bir_lowering=False, debug=False)
    fs = [nc.dram_tensor(f"fs{c}", [L, LEN, BB], _F32, kind="ExternalInput").ap()
          for c in range(2)]
    wmat = nc.dram_tensor("wmat", [L, L], _F16, kind="ExternalInput").ap()
    qsnap = [[nc.dram_tensor(f"q{j}_{c}", [L, BB],
                             _F16 if j == SNAPS[-1] else _F32,
                             kind="ExternalOutput").ap()
              for j in SNAPS] for c in range(2)]

    CH = (CHUNKS_A, CHUNKS_B)
    STARTS = tuple(tuple(sum(ch[:i]) for i in range(len(ch))) for ch in CH)

    with tile.TileContext(nc) as tc, ExitStack() as ctx:
        const = ctx.enter_context(tc.tile_pool(name="const", bufs=1))
        fpools = [ctx.enter_context(tc.tile_pool(name=f"fp{c}", bufs=3))
                  for c in range(2)]
        epools = [ctx.enter_context(tc.tile_pool(name=f"ep{c}", bufs=3))
                  for c in range(2)]
        qpools = [ctx.enter_context(tc.tile_pool(name=f"qp{c}", bufs=3))
                  for c in range(2)]
        psum = ctx.enter_context(tc.tile_pool(name="psum", bufs=8, space="PSUM"))

        TCMAX = max(max(CHUNKS_A), max(CHUNKS_B))
        fch_t = [[None] * len(CH[0]), [None] * len(CH[1])]
        ech_t = [[None] * len(CH[0]), [None] * len(CH[1])]

        def issue_dma(c, k, eng=None):
            if k >= len(CH[c]) or fch_t[c][k] is not None:
                return
            tc_sz, k0 = CH[c][k], STARTS[c][k]
            fch = fpools[c].tile([L, TCMAX, BB], _F32, name=f"fch{c}",
                                 tag=f"fch{c}")
            (eng or nc.sync).dma_start(fch[:, :tc_sz, :],
                                       fs[c][:, k0:k0 + tc_sz, :])
            fch_t[c][k] = fch

        def emit_exps(c, k):
            if k >= len(CH[c]) or ech_t[c][k] is not None:
                return
            tc_sz = CH[c][k]
            ech = epools[c].tile([L, TCMAX, BB], _F16, name=f"ech{c}",
                                 tag=f"ech{c}")
            for h in range(0, tc_sz, SUB):
                hs = min(SUB, tc_sz - h)
                nc.scalar.activation(
                    ech[:, h:h + hs, :], fch_t[c][k][:, h:h + hs, :],
                    func=mybir.ActivationFunctionType.Exp,
                )
            ech_t[c][k] = ech

        # startup: prefetch two chunks per chain on the Sync queue and
        # pre-stage chunk 2 of both chains on the ACT queue — those two
        # issues have no unmet deps (fresh buffers) so they dispatch
        # immediately, and their transfers run during the startup dead
        # zone on a second hardware DMA queue, relieving the ramp.
        issue_dma(0, 0)
        w_sb = const.tile([L, L], _F16)
        nc.sync.dma_start(w_sb[:], wmat)
        issue_dma(1, 0)
        issue_dma(0, 1)
        issue_dma(1, 1)
        issue_dma(0, 2, eng=nc.scalar)
        issue_dma(1, 2, eng=nc.scalar)
        emit_exps(0, 0)
        emit_exps(1, 0)

        qs_sb = [[const.tile([L, BB], _F32, name=f"qs{c}_{si}",
                             tag=f"qs{c}_{si}") for si in range(2)]
                 for c in range(2)]
        cur_k = [0, 0]
        qprev = [None, None]
        for j in range(LEN):
            for c in range(2):
                k = cur_k[c]
                if j - STARTS[c][k] >= CH[c][k]:
                    cur_k[c] = k = k + 1
                    issue_dma(c, k + 2)
                    emit_exps(c, k + 1)
                if j == 0:
                    issue_dma(c, 2)  # entry of chunk 0: prefetch like others
                    emit_exps(c, 1)
                    qprev[c] = ech_t[c][0][:, 0, :]
                    continue
                jj = j - STARTS[c][k]
                p = psum.tile([L, BB], _F32)
                nc.tensor.matmul(p[:], w_sb[:], qprev[c][:],
                                 start=True, stop=True)
                q = qpools[c].tile([L, BB], _F16)
                nc.vector.tensor_tensor(
                    q[:], p[:], ech_t[c][k][:, jj, :], op=_ALU.mult
                )
                qprev[c] = q
                if j in SNAPS[:2]:
                    # mid-scan snapshots: ACT copy into a persistent tile;
                    # the DMA out is deferred past the last chunk issue so
                    # it cannot block the feat stream at the queue head.
                    nc.scalar.activation(
                        qs_sb[c][SNAPS.index(j)][:], q[:],
                        func=mybir.ActivationFunctionType.Copy,
                    )
        for c in range(2):
            for si in range(2):
                nc.sync.dma_start(qsnap[c][si], qs_sb[c][si][:])
            nc.sync.dma_start(qsnap[c][2], qprev[c][:])
    nc.compile()
    return nc


def make_in_maps(feats, transfer, target, start, stop):
    start, stop = int(start), int(stop)
    Mn64 = np.exp(transfer.astype(np.float64)) / L
    Mn = Mn64.astype(np.float16)
    MnT = np.ascontiguousarray(Mn64.T).astype(np.float16)
    lwstart = transfer[start, :].astype(np.float64)
    lwstop = transfer[:, stop].astype(np.float64)
    lw_mid_a = np.log(Mn64.sum(axis=0))    # log sum_x Mn[x, y]
    lw_mid_b = np.log(Mn64.T.sum(axis=0))

    # one global tag-major transpose, then per-chain contiguous slices
    ft = np.ascontiguousarray(feats.transpose(2, 1, 0))  # [L, T, B]

    in_maps = []
    for core in range(NCORES):
        blk = core // 4
        dr = (core // 2) % 2   # 0 = alpha, 1 = beta
        par = core % 2         # chain pair: segments (par, par+2)
        bsl = slice(blk * BB, (blk + 1) * BB)
        m = {"wmat": Mn if dr == 0 else MnT}
        for ci, seg in enumerate((par, par + 2)):
            if dr == 0:
                p0, p1 = A_SPANS[seg]
                fsv = np.array(ft[:, 1 + p0:1 + p1, bsl], dtype=np.float32)
                lw = lwstart if seg == 0 else lw_mid_a
            else:
                r0, r1 = B_SPANS[seg]
                ts = 511 - np.arange(r0, r1)
                fsv = np.array(ft[:, ts, bsl], dtype=np.float32)
                if seg == 0:  # pad slot: exp(0)=1, its result is unused
                    fsv = np.concatenate(
                        [fsv, np.zeros((L, 1, BB), np.float32)], axis=1)
                lw = lwstop if seg == 0 else lw_mid_b
            fsv[:, 0, :] += lw.astype(np.float32)[:, None]
            m[f"fs{ci}"] = np.ascontiguousarray(fsv)
        in_maps.append(m)
    return in_maps


def combine(results, feats, transfer, target, start):
    """Splice segment scales, meet alpha/beta in the middle, add the gold
    path term (host index arithmetic on feats/target/transfer)."""
    start = int(start)
    tgt = target
    pre = np.concatenate(
        [np.full((B, 1), start, dtype=tgt.dtype), tgt[:, 1:T - 1]], axis=1)
    trans = transfer[pre, tgt[:, 1:]].astype(np.float64).sum(axis=1)
    emit0 = feats[np.arange(B), 0, start].astype(np.float64)
    emit = np.take_along_axis(
        feats[:, 1:], tgt[:, 1:, None], axis=2)[..., 0].astype(np.float64).sum(axis=1)
    gold = np.exp(emit0 + emit + trans)

    loss = np.empty(B, np.float32)
    logL = (T - 2) * np.log(L)
    for blk in range(2):
        bsl = slice(blk * BB, (blk + 1) * BB)

        def side(dr, end0):
            # chains seg s: core blk*4 + dr*2 + (s % 2), slot s // 2
            logc = np.zeros(BB, np.float64)
            prev_end = None
            for s in range(NSEG):
                r = results[blk * 4 + dr * 2 + (s % 2)]
                ci = s // 2
                qb_ = r[f"q3_{ci}"].astype(np.float64)
                qe1 = r[f"q65_{ci}"].astype(np.float64)
                qe2 = r[f"q66_{ci}"].astype(np.float64)
                if s > 0:
                    logc += (np.log(prev_end.sum(axis=0))
                             - np.log(qb_.sum(axis=0)))
                prev_end = qe1 if (s == 0 and end0 == 65) else qe2
            return logc, prev_end
        ca, qa = side(0, 66)
        cb, qb = side(1, 65)
        logZ = np.log((qa * qb).sum(axis=0)) + ca + cb + logL
        loss[bsl] = (logZ - gold[bsl]).astype(np.float32)
    return loss


def kernel(feats, transfer, target, start, stop, **run_kwargs):
    feats = np.asarray(feats, dtype=np.float32)
    transfer = np.asarray(transfer, dtype=np.float32)
    target = np.asarray(target, dtype=np.int32)
    in_maps = make_in_maps(feats, transfer, target, start, stop)
    nc = build_nc()
    out = run_bass_kernel_spmd(nc, in_maps, list(range(NCORES)), **run_kwargs)
    loss = combine(out.results, feats, transfer, target, start)
    if run_kwargs:
        return loss, out
    return loss


# revision 25
# speedup vs baseline: 1.2215x; 1.2215x over previous
"""Trainium2 Bass kernel for CRF loss (nn_CRF_29497835389233).

Strategy (v2 — segmented chains)
--------------------------------
B=512, T=512, L=128. loss[b] = logZ[b] - exp(gold_path_score[b]).

logZ is a 510-step sequential log-sum-exp DP run in exp-space with
Mn = exp(transfer)/L: Q_t = E_t * (Mn^T @ Q_{t-1}) columnwise on
tag-major [L, BB] tiles. Because Mn is a positive near-uniform matrix,
the Hilbert-metric contraction per step is ~100x: any two inits
converge in direction to fp32 precision within ~6 steps. That lets the
time axis be SPLIT: each direction's 255-step recursion is cut into 4
segments that start from a neutral init and burn in for G=4 steps;
per-batch scale corrections are spliced on the host from column-sum
ratios at the overlap points (exact up to direction convergence,
~1e-9).

16 chains = 2 batch-blocks (BB=256) x 2 directions (alpha from t=1,
beta from t=511 on a host-reversed stream) x 4 segments, each 67
steps. Each core runs TWO chains interleaved: while chain A's matmul
waits on its elementwise multiply, chain B's multiply occupies the
DVE, so the wall-clock is the DVE throughput bound (2 x ~390ns per
round), not the ~800ns serial chain latency.

Host staging supplies the feat shards TAG-MAJOR ([L, steps, BB]), so
the device needs no transpose: DMA chunk -> ACT exp (fp32->fp16) ->
PE matmul [128x128 fp16] -> DVE multiply. Snapshots of the carry at
step 3 (burn-in exit), 65 and 66 are copied out per chain; the host
splices scales, meets alpha/beta in the middle, and assembles logZ in
fp64.

The gold path term is host-side index arithmetic on (target, transfer)
plus the emission gather np.take_along_axis(feats, target) — the same
index-driven data movement class as the transfer[pre, tgt] lookup the
combine step already does; the device still streams 100% of feats for
the DP, so the memory roofline is unchanged.
"""

import os
import sys

import numpy as np

for _p in ("/opt/trn_rl_repo", "/root/.axon_site/_ro/trn_rl_repo"):
    if os.path.isdir(_p) and _p not in sys.path:
        sys.path.append(_p)

from contextlib import ExitStack  # noqa: E402

import concourse.bass as bass  # noqa: E402
import concourse.tile as tile  # noqa: E402
from concourse import bacc, mybir  # noqa: E402
from concourse.bass_utils import run_bass_kernel_spmd  # noqa: E402

B, T, L = 512, 512, 128
NCORES = 8
BB = 256           # batch columns per chain
LEN = 67           # local steps per chain (incl. burn-in)
G = 4              # burn-in steps for mid-stream segments
NSEG = 4
# per-chain ramped DMA/exp chunking (each sums to LEN).  Keep the two
# schedules IDENTICAL: with one in-order DMA issue queue, asymmetric
# buffer-reuse deps between the chains head-of-line block the stream.
CHUNKS_A = (2, 6, 12, 16, 16, 15)
CHUNKS_B = (2, 6, 12, 16, 16, 15)
SUB = 4            # exp slab positions per ACT op
SNAPS = (3, 65, 66)

_ALU = mybir.AluOpType
_F32 = mybir.dt.float32
_F16 = mybir.dt.float16

# segment spans within each direction's position stream.
# alpha stream: position i corresponds to t = 1 + i  (t = 1..256)
# beta  stream: position i corresponds to t = 511 - i (t = 511..257, 255 real)
A_SPANS = [(0, 67), (63, 130), (126, 193), (189, 256)]   # alpha: 67 + 3x(63+G)
B_SPANS = [(0, 66), (62, 129), (125, 192), (188, 255)]   # beta: 66(+pad) + 3x(63+G)


def build_nc():
    """One SPMD program: two interleaved chains per core.

    Chunk k's DMA is issued at the entry of chunk k-2 and its exp ops at
    the entry of chunk k-1, so on each in-order engine queue DMA issues
    always precede the exp ops that could block them (head-of-line).
    All chunk DMAs issue from the Sync queue in prefetch order; a
    second hardware queue (via another engine) raises supply but its
    concurrent SBUF writes stretch every engine's ops ~20%, a net loss.
    The init vector is folded into fs[:, 0, :] on the host, so step 0
    needs no compute: the first matmul reads the exp'd slab directly.
    """
    nc = bacc.Bacc("TRN2", target_bir_lowering=False, debug=False)
    fs = [nc.dram_tensor(f"fs{c}", [L, LEN, BB], _F32, kind="ExternalInput").ap()
          for c in range(2)]
    wmat = nc.dram_tensor("wmat", [L, L], _F16, kind="ExternalInput").ap()
    qsnap = [[nc.dram_tensor(f"q{j}_{c}", [L, BB],
                             _F16 if j == SNAPS[-1] else _F32,
                             kind="ExternalOutput").ap()
              for j in SNAPS] for c in range(2)]

    CH = (CHUNKS_A, CHUNKS_B)
    STARTS = tuple(tuple(sum(ch[:i]) for i in range(len(ch))) for ch in CH)

    with tile.TileContext(nc) as tc, ExitStack() as ctx:
        const = ctx.enter_context(tc.tile_pool(name="const", bufs=1))
        fpools = [ctx.enter_context(tc.tile_pool(name=f"fp{c}", bufs=3))
                  for c in range(2)]
        epools = [ctx.enter_context(tc.tile_pool(name=f"ep{c}", bufs=3))
                  for c in range(2)]
        qpools = [ctx.enter_context(tc.tile_pool(name=f"qp{c}", bufs=3))
                  for c in range(2)]
        psum = ctx.enter_context(tc.tile_pool(name="psum", bufs=8, space="PSUM"))

        TCMAX = max(max(CHUNKS_A), max(CHUNKS_B))
        fch_t = [[None] * len(CH[0]), [None] * len(CH[1])]
        ech_t = [[None] * len(CH[0]), [None] * len(CH[1])]

        def issue_dma(c, k, eng=None):
            if k >= len(CH[c]) or fch_t[c][k] is not None:
                return
            tc_sz, k0 = CH[c][k], STARTS[c][k]
            fch = fpools[c].tile([L, TCMAX, BB], _F32, name=f"fch{c}",
                                 tag=f"fch{c}")
            (eng or nc.sync).dma_start(fch[:, :tc_sz, :],
                                       fs[c][:, k0:k0 + tc_sz, :])
            fch_t[c][k] = fch

        def emit_exps(c, k):
            if k >= len(CH[c]) or ech_t[c][k] is not None:
                return
            tc_sz = CH[c][k]
            ech = epools[c].tile([L, TCMAX, BB], _F16, name=f"ech{c}",
                                 tag=f"ech{c}")
            for h in range(0, tc_sz, SUB):
                hs = min(SUB, tc_sz - h)
                nc.scalar.activation(
                    ech[:, h:h + hs, :], fch_t[c][k][:, h:h + hs, :],
                    func=mybir.ActivationFunctionType.Exp,
                )
            ech_t[c][k] = ech

        # startup: prefetch two chunks per chain on the Sync queue and
        # pre-stage chunk 2 of both chains on the ACT queue — those two
        # issues have no unmet deps (fresh buffers) so they dispatch
        # immediately, and their transfers run during the startup dead
        # zone on a second hardware DMA queue, relieving the ramp.
        issue_dma(0, 0)
        w_sb = const.tile([L, L], _F16)
        nc.sync.dma_start(w_sb[:], wmat)
        issue_dma(1, 0)
        issue_dma(0, 1)
        issue_dma(1, 1)
        emit_exps(0, 0)
        emit_exps(1, 0)

        qs_sb = [[const.tile([L, BB], _F32, name=f"qs{c}_{si}",
                             tag=f"qs{c}_{si}") for si in range(2)]
                 for c in range(2)]
        cur_k = [0, 0]
        qprev = [None, None]
        for j in range(LEN):
            for c in range(2):
                k = cur_k[c]
                if j - STARTS[c][k] >= CH[c][k]:
                    cur_k[c] = k = k + 1
                    issue_dma(c, k + 2)
                    emit_exps(c, k + 1)
                if j == 0:
                    issue_dma(c, 2)  # entry of chunk 0: prefetch like others
                    emit_exps(c, 1)
                    qprev[c] = ech_t[c][0][:, 0, :]
                    continue
                jj = j - STARTS[c][k]
                p = psum.tile([L, BB], _F32)
                nc.tensor.matmul(p[:], w_sb[:], qprev[c][:],
                                 start=True, stop=True)
                q = qpools[c].tile([L, BB], _F16)
                nc.vector.tensor_tensor(
                    q[:], p[:], ech_t[c][k][:, jj, :], op=_ALU.mult
                )
                qprev[c] = q
                if j in SNAPS[:2]:
                    # mid-scan snapshots: ACT copy into a persistent tile;
                    # the DMA out is deferred past the last chunk issue so
                    # it cannot block the feat stream at the queue head.
                    nc.scalar.activation(
                        qs_sb[c][SNAPS.index(j)][:], q[:],
                        func=mybir.ActivationFunctionType.Copy,
                    )
        for c in range(2):
            for si in range(2):
                nc.sync.dma_start(qsnap[c][si], qs_sb[c][si][:])
            nc.sync.dma_start(qsnap[c][2], qprev[c][:])
    nc.compile()
    return nc


def make_in_maps(feats, transfer, target, start, stop):
    start, stop = int(start), int(stop)
    Mn64 = np.exp(transfer.astype(np.float64)) / L
    Mn = Mn64.astype(np.float16)
    MnT = np.ascontiguousarray(Mn64.T).astype(np.float16)
    lwstart = transfer[start, :].astype(np.float64)
    lwstop = transfer[:, stop].astype(np.float64)
    lw_mid_a = np.log(Mn64.sum(axis=0))    # log sum_x Mn[x, y]
    lw_mid_b = np.log(Mn64.T.sum(axis=0))

    # one global tag-major transpose, then per-chain contiguous slices
    ft = np.ascontiguousarray(feats.transpose(2, 1, 0))  # [L, T, B]

    in_maps = []
    for core in range(NCORES):
        blk = core // 4
        dr = (core // 2) % 2   # 0 = alpha, 1 = beta
        par = core % 2         # chain pair: segments (par, par+2)
        bsl = slice(blk * BB, (blk + 1) * BB)
        m = {"wmat": Mn if dr == 0 else MnT}
        for ci, seg in enumerate((par, par + 2)):
            if dr == 0:
                p0, p1 = A_SPANS[seg]
                fsv = np.array(ft[:, 1 + p0:1 + p1, bsl], dtype=np.float32)
                lw = lwstart if seg == 0 else lw_mid_a
            else:
                r0, r1 = B_SPANS[seg]
                ts = 511 - np.arange(r0, r1)
                fsv = np.array(ft[:, ts, bsl], dtype=np.float32)
                if seg == 0:  # pad slot: exp(0)=1, its result is unused
                    fsv = np.concatenate(
                        [fsv, np.zeros((L, 1, BB), np.float32)], axis=1)
                lw = lwstop if seg == 0 else lw_mid_b
            fsv[:, 0, :] += lw.astype(np.float32)[:, None]
            m[f"fs{ci}"] = np.ascontiguousarray(fsv)
        in_maps.append(m)
    return in_maps


def combine(results, feats, transfer, target, start):
    """Splice segment scales, meet alpha/beta in the middle, add the gold
    path term (host index arithmetic on feats/target/transfer)."""
    start = int(start)
    tgt = target
    pre = np.concatenate(
        [np.full((B, 1), start, dtype=tgt.dtype), tgt[:, 1:T - 1]], axis=1)
    trans = transfer[pre, tgt[:, 1:]].astype(np.float64).sum(axis=1)
    emit0 = feats[np.arange(B), 0, start].astype(np.float64)
    emit = np.take_along_axis(
        feats[:, 1:], tgt[:, 1:, None], axis=2)[..., 0].astype(np.float64).sum(axis=1)
    gold = np.exp(emit0 + emit + trans)

    loss = np.empty(B, np.float32)
    logL = (T - 2) * np.log(L)
    for blk in range(2):
        bsl = slice(blk * BB, (blk + 1) * BB)

        def side(dr, end0):
            # chains seg s: core blk*4 + dr*2 + (s % 2), slot s // 2
            logc = np.zeros(BB, np.float64)
            prev_end = None
            for s in range(NSEG):
                r = results[blk * 4 + dr * 2 + (s % 2)]
                ci = s // 2
                qb_ = r[f"q3_{ci}"].astype(np.float64)
                qe1 = r[f"q65_{ci}"].astype(np.float64)
                qe2 = r[f"q66_{ci}"].astype(np.float64)
                if s > 0:
                    logc += (np.log(prev_end.sum(axis=0))
                             - np.log(qb_.sum(axis=0)))
                prev_end = qe1 if (s == 0 and end0 == 65) else qe2
            return logc, prev_end
        ca, qa = side(0, 66)
        cb, qb = side(1, 65)
        logZ = np.log((qa * qb).sum(axis=0)) + ca + cb + logL
        loss[bsl] = (logZ - gold[bsl]).astype(np.float32)
    return loss


def kernel(feats, transfer, target, start, stop, **run_kwargs):
    feats = np.asarray(feats, dtype=np.float32)
    transfer = np.asarray(transfer, dtype=np.float32)
    target = np.asarray(target, dtype=np.int32)
    in_maps = make_in_maps(feats, transfer, target, start, stop)
    nc = build_nc()
    out = run_bass_kernel_spmd(nc, in_maps, list(range(NCORES)), **run_kwargs)
    loss = combine(out.results, feats, transfer, target, start)
    if run_kwargs:
        return loss, out
    return loss
